# revision 2
# baseline (speedup 1.0000x reference)
"""Trainium2 Bass kernel for a chain of 20 radial flows on [8388608, 4] fp32.

Strategy: pure data parallel over 8 NeuronCores. Each core gets B/8 samples,
host-transposed to [4, S] so components sit on separate SBUF partitions
(partition 32*c + b holds component c of sample-block b).

Per flow k (sequential chain):
  d   = v + delta_{k-1}            (delta folded as per-partition ACT bias)
  sq  = d*d                        ACT Square (bf16 out), [128, 4096] chunks
  r2  = sum_c sq[c]                PE matmul with 0/1 bf16 stationary,
                                   packed onto partition quadrants, [512] PSUM
  r   = sqrt(r2)                   ACT, 4 tiles -> one [128, 2048] SBUF tile
  w   = r/beta + alpha/beta        GPSIMD tensor_scalar (idle engine; per-flow
                                   constants baked as immediates)
  m   = 1 + 1/w                    custom DVE op (RADIAL_M): quadratic
                                   Chebyshev seed on x*bitcast(~x), +1 via
                                   a per-partition ones tile (~5e-5 rel)
  m_b = broadcast(m)               PE matmul with 0/1 stationary -> PSUM,
                                   [1536]-col tiles (3 PSUM banks) x2 bufs
  v  <- (v + delta_{k-1}) * m_b    DVE scalar_tensor_tensor
Final: out = v + x0[19] (ACT Identity bias), DMA out per chunk.
"""

import sys

if "/opt/trn_rl_repo" not in sys.path:
    sys.path.insert(0, "/opt/trn_rl_repo")

from contextlib import ExitStack

import numpy as np

import concourse.bass as bass
import concourse.tile as tile
from concourse import bacc, mybir

F32 = mybir.dt.float32
F32R = mybir.dt.float32r
BF16 = mybir.dt.bfloat16

BATCH = 8388608
DIM = 4
N_FLOWS = 20
N_CORES = 8
S = BATCH // N_CORES          # samples per core
NB = 32                       # sample-blocks (per quadrant of partitions)
FC = 4096                     # square-chunk free-dim (columns)
NT = S // NB // FC            # square chunks per core (8)
COLS = S // NB                # total state columns per core (32768)
MT = 1536                     # STT/mb tile columns (3 PSUM banks)

_CACHE = {}

# Chebyshev-minimax quadratic seed for 1/x on the u = x*bitcast(~x) interval
# [-4.5, -4]: 1/x ~= bitcast(~x) * (c2*u^2 + c1*u + c0), max rel err 5.2e-5.
_RM_C = (-0.706758876, -0.166355887, -0.013040724)


def _radial_m_op():
    """out = bitcast(~x)*((C2*u + C1)*u + C0) + src1, u = x*bitcast(~x).

    With src1 = ones column: out = 1 + 1/Src0 to ~5e-5 — fuses the
    reciprocal and the +1 of m = 1 + beta/(alpha+r) into one 8-stage op."""
    from concourse import dve_ops
    from concourse.dve_spec import AluOp, Bin, C0, C1, C2, Spec, Src0, Src1, lower
    from concourse.dve_uop import DveOpSpec

    for op in dve_ops.OPS:
        if op.name == "RADIAL_M":
            return op
    _not = Bin(AluOp.BITWISE_NOT, Src0, Src0)
    _u = Src0 * _not
    body = _not * ((_u * C2 + C1) * _u + C0) + Src1

    def ref(in0, in1, c0, c1, c2):
        not_x = (~in0.view(np.int32)).view(np.float32)
        u = (in0 * not_x).astype(np.float32)
        return (not_x * ((u * c2 + c1) * u + c0) + in1).astype(np.float32)

    spec = Spec(body=body, reference=ref)
    row = max(dve_ops._SUB_OPCODE_FOR_NAME.values()) + 1
    assert row < 0x20
    dve_ops._SUB_OPCODE_FOR_NAME["RADIAL_M"] = row
    shas = {}
    for ver in ("v3", "v4"):
        uops = lower(spec, ver=ver)
        shas[ver] = DveOpSpec(
            name="RADIAL_M", opcode=row, uops=uops, rd1_en=True).sha(ver)
    op = dve_ops.DveOp("RADIAL_M", spec, subdim=False, uops_sha=shas)
    dve_ops.OPS.append(op)
    dve_ops.CUSTOM_DVE_SPECS["RADIAL_M"] = spec
    return op


def _flow_consts(alpha_primes, beta_primes):
    sp_a = np.logaddexp(np.float32(0.0), np.asarray(alpha_primes, np.float32))
    sp_b = np.logaddexp(np.float32(0.0), np.asarray(beta_primes, np.float32))
    alpha = sp_a.astype(np.float32)
    beta = (-alpha + sp_b).astype(np.float32)
    beta = np.where(beta == 0.0, np.float32(1e-30), beta)
    return alpha, beta


def _build_program(alpha=None, beta=None, n_flows=N_FLOWS, nt=NT):
    if alpha is None:
        alpha = np.ones(N_FLOWS, np.float32)
        beta = np.ones(N_FLOWS, np.float32)
    nc = bacc.Bacc("TRN2", target_bir_lowering=False, debug=False,
                   num_devices=N_CORES)
    s = nt * NB * FC
    cols = nt * FC
    xt = nc.dram_tensor("xt", [DIM, s], F32, kind="ExternalInput").ap()
    wr = nc.dram_tensor("wr", [4, 128, 128], BF16, kind="ExternalInput").ap()
    wb = nc.dram_tensor("wb", [4, 128, 128], F32R, kind="ExternalInput").ap()
    pr = nc.dram_tensor("pr", [128, N_FLOWS + 1], F32,
                        kind="ExternalInput").ap()
    ot = nc.dram_tensor("ot", [DIM, s], F32, kind="ExternalOutput").ap()

    # DRAM view: partition (c*32+b) <- comp c, block b; free (t, f)
    xt_r = xt.rearrange("c (b t f) -> (c b) t f", b=NB, f=FC)
    ot_r = ot.rearrange("c (b t f) -> (c b) t f", b=NB, f=FC)

    AL = mybir.AluOpType
    ACT = mybir.ActivationFunctionType

    # STT tile column offsets/sizes per flow: 21 x 1536 + 1 x 512
    stt_tiles = []
    c0 = 0
    while c0 < cols:
        L = min(MT, cols - c0)
        stt_tiles.append((c0, L))
        c0 += L

    def col(i):  # per-partition scalar AP from the params tile
        return pr_t[:, i:i + 1]

    with tile.TileContext(nc) as tc, ExitStack() as ctx:
        singles = ctx.enter_context(tc.tile_pool(name="singles", bufs=1))
        state = ctx.enter_context(tc.tile_pool(name="state", bufs=1))
        sq_pool = ctx.enter_context(tc.tile_pool(name="sq", bufs=2))
        r_pool = ctx.enter_context(tc.tile_pool(name="r", bufs=2))
        m_pool = ctx.enter_context(tc.tile_pool(name="m", bufs=2))
        r2_pool = ctx.enter_context(
            tc.tile_pool(name="r2", bufs=2, space="PSUM"))
        mb_pool = ctx.enter_context(
            tc.tile_pool(name="mb", bufs=2, space="PSUM"))

        pr_t = singles.tile([128, N_FLOWS + 1], F32)
        nc.sync.dma_start(pr_t[:], pr[:])
        wr_t = singles.tile([128, 4, 128], BF16)
        nc.sync.dma_start(wr_t[:], wr.rearrange("j k m -> k j m"))
        wb_t = singles.tile([128, 4, 128], F32R)
        nc.sync.dma_start(wb_t[:], wb.rearrange("j k m -> k j m"))
        ones_t = singles.tile([128, 2048], F32)
        nc.vector.memset(ones_t[:], 1.0)

        v = state.tile([128, cols], F32)
        for t in range(nt):
            nc.sync.dma_start(v[:, bass.ts(t, FC)], xt_r[:, t, :])

        for k in range(n_flows):
            inv_b = float(1.0 / beta[k])
            a_ov_b = float(alpha[k] / beta[k])
            # --- squares (ACT) + packed r2 (PE) + sqrt (ACT) -> r tiles ---
            sq_tiles = {}
            for t in range(nt):
                sq = sq_pool.tile([128, FC], BF16)
                nc.scalar.activation(sq[:], v[:, bass.ts(t, FC)], ACT.Square,
                                     bias=col(k), scale=1.0)
                sq_tiles[t] = sq
            # one m/r/w tile per 8192 state cols (4 r2 tiles of 512)
            m_tiles = {}
            for q in range(cols // 8192):
                r = r_pool.tile([128, 2048], F32, tag="r")
                for uu in range(4):
                    u = 4 * q + uu       # r2 tile index, covers 2048 cols
                    scol = 2048 * u
                    t = scol // FC
                    base = scol % FC
                    r2 = r2_pool.tile([128, 512], F32)
                    for j in range(4):
                        nc.tensor.matmul(
                            out=r2[:],
                            lhsT=wr_t[:, j, :],
                            rhs=sq_tiles[t][:, base + 512 * j:
                                            base + 512 * (j + 1)],
                            start=(j == 0), stop=(j == 3))
                    nc.scalar.activation(r[:, bass.ts(uu, 512)], r2[:],
                                         ACT.Sqrt)
                # w = r/beta + alpha/beta on GPSIMD (idle engine), in place
                nc.gpsimd.tensor_scalar(out=r[:], in0=r[:], scalar1=inv_b,
                                        scalar2=a_ov_b, op0=AL.mult,
                                        op1=AL.add)
                m = m_pool.tile([128, 2048], F32R, tag="m")
                nc.vector._custom_dve(
                    _radial_m_op(), out=m[:], in0=r[:],
                    in1=ones_t[:], s0=_RM_C[0], s1=_RM_C[1],
                    imm2=_RM_C[2])
                m_tiles[q] = m

            # --- broadcast (PE) + state update (DVE STT) ---
            for (c0, L) in stt_tiles:
                mb = mb_pool.tile([128, MT], F32)
                for i in range(L // 512):
                    sc = c0 + 512 * i
                    q = sc // 8192
                    uu = (sc // 2048) % 4
                    j = (sc % 2048) // 512
                    nc.tensor.matmul(
                        out=mb[:, bass.ts(i, 512)],
                        lhsT=wb_t[:, j, :],
                        rhs=m_tiles[q][:, bass.ts(uu, 512)],
                        start=True, stop=True)
                vs = v[:, c0:c0 + L]
                nc.vector.scalar_tensor_tensor(
                    out=vs, in0=vs, scalar=col(k), in1=mb[:, 0:L],
                    op0=AL.add, op1=AL.mult)

            if k == n_flows - 1:
                for t in range(nt):
                    vchunk = v[:, bass.ts(t, FC)]
                    nc.scalar.activation(vchunk, vchunk, ACT.Identity,
                                         bias=col(N_FLOWS), scale=1.0)
                    nc.sync.dma_start(ot_r[:, t, :], vchunk)

    nc.compile()
    return nc


def _host_params(x0s, alpha_primes, beta_primes, n_flows=N_FLOWS):
    x0s = np.asarray(x0s, np.float32)
    alpha, beta = _flow_consts(alpha_primes, beta_primes)

    # params: dprev[k] (k=0..19), final
    pr = np.zeros((128, N_FLOWS + 1), np.float32)
    comp = np.arange(128) // 32  # component index per partition
    for k in range(n_flows):
        dprev = -x0s[0] if k == 0 else x0s[k - 1] - x0s[k]
        pr[:, k] = dprev[comp]
    pr[:, N_FLOWS] = x0s[n_flows - 1][comp]

    # stationaries: wr reduce (comp partitions -> packed), wb broadcast
    import ml_dtypes
    wr = np.zeros((4, 128, 128), np.float32)
    wb = np.zeros((4, 128, 128), np.float32)
    b = np.arange(NB)
    for j in range(4):
        for c in range(4):
            wr[j, 32 * c + b, 32 * j + b] = 1.0
            wb[j, 32 * j + b, 32 * c + b] = 1.0
    return pr, wr.astype(ml_dtypes.bfloat16), wb


def kernel(X, x0s, alpha_primes, beta_primes):
    from concourse.bass_utils import run_bass_kernel_spmd

    X = np.asarray(X, np.float32)
    pr, wr, wb = _host_params(x0s, alpha_primes, beta_primes)
    alpha, beta = _flow_consts(alpha_primes, beta_primes)

    key = (alpha.tobytes(), beta.tobytes())
    if _CACHE.get("key") != key:
        _CACHE["nc"] = _build_program(alpha, beta)
        _CACHE["key"] = key
    nc = _CACHE["nc"]

    in_maps = []
    for c in range(N_CORES):
        shard = X[c * S:(c + 1) * S]
        in_maps.append({
            "xt": np.ascontiguousarray(shard.T),
            "wr": wr,
            "wb": wb,
            "pr": pr,
        })
    # The device occasionally throws a spurious NRT_EXEC_UNIT_UNRECOVERABLE
    # and recovers by the next run; retry rather than failing the call.
    res = None
    for attempt in range(3):
        try:
            res = run_bass_kernel_spmd(nc, in_maps, list(range(N_CORES)))
            break
        except Exception:
            if attempt == 2:
                raise
    out = np.empty((BATCH, DIM), np.float32)
    for c in range(N_CORES):
        out[c * S:(c + 1) * S] = res.results[c]["ot"].T
    return out


# revision 5
# speedup vs baseline: 1.3508x; 1.3508x over previous
"""Trainium2 Bass kernel for a chain of 20 radial flows on [8388608, 4] fp32.

Strategy: pure data parallel over 8 NeuronCores. Each core gets B/8 samples,
host-transposed to [4, S] so components sit on separate SBUF partitions
(partition 32*c + b holds component c of sample-block b).

Per flow k (sequential chain):
  d   = v + delta_{k-1}            (delta folded as per-partition ACT bias)
  sq  = d*d                        ACT Square (bf16 out), [128, 4096] chunks
  r2  = sum_c sq[c]                PE matmul with 0/1 bf16 stationary,
                                   packed onto partition quadrants, [512] PSUM
  r   = sqrt(r2)                   ACT, 4 tiles -> one [128, 2048] SBUF tile
  w   = r/beta + alpha/beta        GPSIMD tensor_scalar (idle engine; per-flow
                                   constants baked as immediates)
  m   = 1 + 1/w                    custom DVE op (RADIAL_M): quadratic
                                   Chebyshev seed on x*bitcast(~x), +1 via
                                   a per-partition ones tile (~5e-5 rel)
  m_b = broadcast(m)               PE matmul with 0/1 stationary -> PSUM,
                                   [1536]-col tiles (3 PSUM banks) x2 bufs
  v  <- (v + delta_{k-1}) * m_b    DVE scalar_tensor_tensor
Final: out = v + x0[19] (ACT Identity bias), DMA out per chunk.
"""

import sys

if "/opt/trn_rl_repo" not in sys.path:
    sys.path.insert(0, "/opt/trn_rl_repo")

from contextlib import ExitStack

import numpy as np

import concourse.bass as bass
import concourse.tile as tile
from concourse import bacc, mybir

F32 = mybir.dt.float32
F32R = mybir.dt.float32r
BF16 = mybir.dt.bfloat16

BATCH = 8388608
DIM = 4
N_FLOWS = 20
N_CORES = 8
S = BATCH // N_CORES          # samples per core
NB = 32                       # sample-blocks (per quadrant of partitions)
FC = 4096                     # square-chunk free-dim (columns)
NT = S // NB // FC            # square chunks per core (8)
COLS = S // NB                # total state columns per core (32768)
MT = 1536                     # STT/mb tile columns (3 PSUM banks)

_CACHE = {}

# Chebyshev-minimax quadratic seed for 1/x on the u = x*bitcast(~x) interval
# [-4.5, -4]: 1/x ~= bitcast(~x) * (c2*u^2 + c1*u + c0), max rel err 5.2e-5.
_RM_C = (-0.706758876, -0.166355887, -0.013040724)


def _radial_m_op():
    """out = bitcast(~x)*((C2*u + C1)*u + C0) + src1, u = x*bitcast(~x).

    With src1 = ones column: out = 1 + 1/Src0 to ~5e-5 — fuses the
    reciprocal and the +1 of m = 1 + beta/(alpha+r) into one 8-stage op."""
    from concourse import dve_ops
    from concourse.dve_spec import AluOp, Bin, C0, C1, C2, Spec, Src0, Src1, lower
    from concourse.dve_uop import DveOpSpec

    for op in dve_ops.OPS:
        if op.name == "RADIAL_M":
            return op
    _not = Bin(AluOp.BITWISE_NOT, Src0, Src0)
    _u = Src0 * _not
    body = _not * ((_u * C2 + C1) * _u + C0) + Src1

    def ref(in0, in1, c0, c1, c2):
        not_x = (~in0.view(np.int32)).view(np.float32)
        u = (in0 * not_x).astype(np.float32)
        return (not_x * ((u * c2 + c1) * u + c0) + in1).astype(np.float32)

    spec = Spec(body=body, reference=ref)
    row = max(dve_ops._SUB_OPCODE_FOR_NAME.values()) + 1
    assert row < 0x20
    dve_ops._SUB_OPCODE_FOR_NAME["RADIAL_M"] = row
    shas = {}
    for ver in ("v3", "v4"):
        uops = lower(spec, ver=ver)
        shas[ver] = DveOpSpec(
            name="RADIAL_M", opcode=row, uops=uops, rd1_en=True).sha(ver)
    op = dve_ops.DveOp("RADIAL_M", spec, subdim=False, uops_sha=shas)
    dve_ops.OPS.append(op)
    dve_ops.CUSTOM_DVE_SPECS["RADIAL_M"] = spec
    return op


def _flow_consts(alpha_primes, beta_primes):
    sp_a = np.logaddexp(np.float32(0.0), np.asarray(alpha_primes, np.float32))
    sp_b = np.logaddexp(np.float32(0.0), np.asarray(beta_primes, np.float32))
    alpha = sp_a.astype(np.float32)
    beta = (-alpha + sp_b).astype(np.float32)
    beta = np.where(beta == 0.0, np.float32(1e-30), beta)
    return alpha, beta


def _build_program(alpha=None, beta=None, n_flows=N_FLOWS, nt=NT):
    if alpha is None:
        alpha = np.ones(N_FLOWS, np.float32)
        beta = np.ones(N_FLOWS, np.float32)
    nc = bacc.Bacc("TRN2", target_bir_lowering=False, debug=False,
                   num_devices=N_CORES)
    s = nt * NB * FC
    cols = nt * FC
    xt = nc.dram_tensor("xt", [DIM, s], F32, kind="ExternalInput").ap()
    wr = nc.dram_tensor("wr", [4, 128, 128], BF16, kind="ExternalInput").ap()
    wb = nc.dram_tensor("wb", [4, 128, 128], F32R, kind="ExternalInput").ap()
    pr = nc.dram_tensor("pr", [128, N_FLOWS + 1], F32,
                        kind="ExternalInput").ap()
    ot = nc.dram_tensor("ot", [DIM, s], F32, kind="ExternalOutput").ap()

    # DRAM view: partition (c*32+b) <- comp c, block b; free (t, f)
    xt_r = xt.rearrange("c (b t f) -> (c b) t f", b=NB, f=FC)
    ot_r = ot.rearrange("c (b t f) -> (c b) t f", b=NB, f=FC)

    AL = mybir.AluOpType
    ACT = mybir.ActivationFunctionType

    # STT tiles, aligned to 8192-col m-groups: per group 5 x 1536 + 1 x 512
    stt_tiles = []
    for g in range(cols // 8192):
        c0 = 8192 * g
        while c0 < 8192 * (g + 1):
            L = min(MT, 8192 * (g + 1) - c0)
            stt_tiles.append((c0, L))
            c0 += L

    def col(i):  # per-partition scalar AP from the params tile
        return pr_t[:, i:i + 1]

    with tile.TileContext(nc) as tc, ExitStack() as ctx:
        singles = ctx.enter_context(tc.tile_pool(name="singles", bufs=1))
        state = ctx.enter_context(tc.tile_pool(name="state", bufs=1))
        sq_pool = ctx.enter_context(tc.tile_pool(name="sq", bufs=2))
        r_pool = ctx.enter_context(tc.tile_pool(name="r", bufs=2))
        m_pool = ctx.enter_context(tc.tile_pool(name="m", bufs=2))
        r2_pool = ctx.enter_context(
            tc.tile_pool(name="r2", bufs=2, space="PSUM"))
        mb_pool = ctx.enter_context(
            tc.tile_pool(name="mb", bufs=2, space="PSUM"))

        pr_t = singles.tile([128, N_FLOWS + 1], F32)
        nc.sync.dma_start(pr_t[:], pr[:])
        wr_t = singles.tile([128, 4, 128], BF16)
        nc.sync.dma_start(wr_t[:], wr.rearrange("j k m -> k j m"))
        wb_t = singles.tile([128, 4, 128], F32R)
        nc.sync.dma_start(wb_t[:], wb.rearrange("j k m -> k j m"))
        ones_t = singles.tile([128, 2048], F32)
        nc.vector.memset(ones_t[:], 1.0)

        v = state.tile([128, cols], F32)
        for t in range(nt):
            nc.sync.dma_start(v[:, bass.ts(t, FC)], xt_r[:, t, :])

        NQ = cols // 8192       # m-groups per flow (4)
        # software pipeline with one-group lag: the m-chain for group (k, q)
        # is emitted before the STT batch of group (k, q-1), so PE always has
        # broadcast matmuls ready while the DVE drains the previous batch.
        stages = []             # (k, q) m-chain stages in emission order
        for k in range(n_flows):
            for q in range(NQ):
                stages.append((k, q))

        m_tiles = {}            # (k, q) -> m tile

        def emit_mchain(k, q):
            inv_b = float(1.0 / beta[k])
            a_ov_b = float(alpha[k] / beta[k])
            # squares for the two 4096-chunks of this group
            sq_tiles = []
            for h in range(2):
                t = 2 * q + h
                sq = sq_pool.tile([128, FC], BF16)
                nc.scalar.activation(sq[:], v[:, bass.ts(t, FC)], ACT.Square,
                                     bias=col(k), scale=1.0)
                sq_tiles.append(sq)
            r = r_pool.tile([128, 2048], F32, tag="r")
            for uu in range(4):
                scol = 2048 * uu
                r2 = r2_pool.tile([128, 512], F32)
                for j in range(4):
                    nc.tensor.matmul(
                        out=r2[:],
                        lhsT=wr_t[:, j, :],
                        rhs=sq_tiles[scol // FC][:, (scol % FC) + 512 * j:
                                                 (scol % FC) + 512 * (j + 1)],
                        start=(j == 0), stop=(j == 3))
                nc.scalar.activation(r[:, bass.ts(uu, 512)], r2[:], ACT.Sqrt)
            # w = r/beta + alpha/beta on GPSIMD (idle engine), in place
            nc.gpsimd.tensor_scalar(out=r[:], in0=r[:], scalar1=inv_b,
                                    scalar2=a_ov_b, op0=AL.mult, op1=AL.add)
            m = m_pool.tile([128, 2048], F32R, tag="m")
            nc.vector._custom_dve(
                _radial_m_op(), out=m[:], in0=r[:],
                in1=ones_t[:], s0=_RM_C[0], s1=_RM_C[1], imm2=_RM_C[2])
            m_tiles[(k, q)] = m

        def emit_stt_batch(k, q):
            # STT tiles whose last column falls in group q
            for (c0, L) in stt_tiles:
                if (c0 + L - 1) // 8192 != q:
                    continue
                mb = mb_pool.tile([128, MT], F32)
                for i in range(L // 512):
                    sc = c0 + 512 * i
                    nc.tensor.matmul(
                        out=mb[:, bass.ts(i, 512)],
                        lhsT=wb_t[:, (sc % 2048) // 512, :],
                        rhs=m_tiles[(k, sc // 8192)][:, bass.ts(
                            (sc // 2048) % 4, 512)],
                        start=True, stop=True)
                vs = v[:, c0:c0 + L]
                nc.vector.scalar_tensor_tensor(
                    out=vs, in0=vs, scalar=col(k), in1=mb[:, 0:L],
                    op0=AL.add, op1=AL.mult)

        prev = None
        for (k, q) in stages:
            emit_mchain(k, q)
            if prev is not None:
                emit_stt_batch(*prev)
                m_tiles.pop(prev)
            prev = (k, q)
        emit_stt_batch(*prev)

        for t in range(nt):
            vchunk = v[:, bass.ts(t, FC)]
            nc.scalar.activation(vchunk, vchunk, ACT.Identity,
                                 bias=col(N_FLOWS), scale=1.0)
            nc.sync.dma_start(ot_r[:, t, :], vchunk)

    nc.compile()
    return nc


def _host_params(x0s, alpha_primes, beta_primes, n_flows=N_FLOWS):
    x0s = np.asarray(x0s, np.float32)
    alpha, beta = _flow_consts(alpha_primes, beta_primes)

    # params: dprev[k] (k=0..19), final
    pr = np.zeros((128, N_FLOWS + 1), np.float32)
    comp = np.arange(128) // 32  # component index per partition
    for k in range(n_flows):
        dprev = -x0s[0] if k == 0 else x0s[k - 1] - x0s[k]
        pr[:, k] = dprev[comp]
    pr[:, N_FLOWS] = x0s[n_flows - 1][comp]

    # stationaries: wr reduce (comp partitions -> packed), wb broadcast
    import ml_dtypes
    wr = np.zeros((4, 128, 128), np.float32)
    wb = np.zeros((4, 128, 128), np.float32)
    b = np.arange(NB)
    for j in range(4):
        for c in range(4):
            wr[j, 32 * c + b, 32 * j + b] = 1.0
            wb[j, 32 * j + b, 32 * c + b] = 1.0
    return pr, wr.astype(ml_dtypes.bfloat16), wb


def kernel(X, x0s, alpha_primes, beta_primes):
    from concourse.bass_utils import run_bass_kernel_spmd

    X = np.asarray(X, np.float32)
    pr, wr, wb = _host_params(x0s, alpha_primes, beta_primes)
    alpha, beta = _flow_consts(alpha_primes, beta_primes)

    key = (alpha.tobytes(), beta.tobytes())
    if _CACHE.get("key") != key:
        _CACHE["nc"] = _build_program(alpha, beta)
        _CACHE["key"] = key
    nc = _CACHE["nc"]

    in_maps = []
    for c in range(N_CORES):
        shard = X[c * S:(c + 1) * S]
        in_maps.append({
            "xt": np.ascontiguousarray(shard.T),
            "wr": wr,
            "wb": wb,
            "pr": pr,
        })
    # The device occasionally throws a spurious NRT_EXEC_UNIT_UNRECOVERABLE
    # and recovers by the next run; retry rather than failing the call.
    res = None
    for attempt in range(3):
        try:
            res = run_bass_kernel_spmd(nc, in_maps, list(range(N_CORES)))
            break
        except Exception:
            if attempt == 2:
                raise
    out = np.empty((BATCH, DIM), np.float32)
    for c in range(N_CORES):
        out[c * S:(c + 1) * S] = res.results[c]["ot"].T
    return out
